# revision 3
# baseline (speedup 1.0000x reference)
"""Multi-head attention layer on 8 Trainium2 NeuronCores.

Reference (per batch n):
    Q = x@Wq + bq; K = x@Wk + bk; V = x@Wv + bv       (per-head split, Dh=64)
    out = softmax(Q K^T / sqrt(Dh)) V  -> concat heads -> @Wo + bo

Sharding: 2 head-groups (tensor parallel) x 4 batches (data parallel) = 8
cores. Core c handles batch c%4 and heads [8*(c//4), 8*(c//4)+8). Each core
computes a partial output projection with its Wo row-block; the host sums
the two head-group partials per batch (the only cross-core reduction).

Per-core kernel (all matmuls in float32r = TF32-like, fp32 accumulate;
output projection in bf16):
  A) projections: K^T,Q^T in [d_head(part), seq] layout, V in [seq(part),
     d_head] layout with an appended ones column (gives softmax denominators
     for free), streaming x^T per 512-column chunk.
  B) attention per head: S^T = K Q^T on TensorE (row-group 0/64 per head
     parity), exp via ScalarE over [128,1024] PSUM tiles (scale=1/8, no max
     subtraction -- scores are O(1) by construction), O^T += V_aug^T exp(S^T)
     accumulated over 16 seq-tiles, then normalize by the broadcast
     reciprocal of the denominator row (K=1 ones matmul broadcast).
  C) out^T partial = O_norm^T-contracted with Wo rows + bo (g==0 only).

Self-contained: hardcodes shapes for x:[4,2048,1024], d_model=1024, 16 heads.
"""

import sys
import types
import contextlib

import numpy as np

import concourse.bass as bass
import concourse.mybir as mybir
import concourse.tile as tile
from concourse import bacc
from concourse.bass_utils import run_bass_kernel_spmd

f32 = mybir.dt.float32
f32r = mybir.dt.float32r
bf16 = mybir.dt.bfloat16
AF = mybir.ActivationFunctionType

N_CORES = 8
P = 128

# ---------------------------------------------------------------------------


def build_nc(L=2048, D=1024, HPC=8, Dh=64):
    """Build the per-core Bass graph (SPMD: same graph, per-core shards)."""
    KO = D // P          # k-tiles over d_model
    DQ = HPC * Dh        # local projected dim
    DKC = DQ // P        # 128-row chunks of DQ
    NSC = L // 512       # 512-wide seq chunks
    ST = L // P          # 128-row seq tiles
    MC = L // 512        # 512-wide m chunks
    WKO = DQ // P        # k-tiles for out-proj contraction
    EC = D // 512        # 512-wide out chunks
    MS = L // P          # 128-row out row-tiles
    assert MC % 2 == 0 and HPC % 2 == 0

    nc = bacc.Bacc("TRN2", target_bir_lowering=False, debug=False,
                   num_devices=N_CORES)

    xT_d = nc.dram_tensor("xT", [D, L], f32r, kind="ExternalInput")
    Wq_d = nc.dram_tensor("Wq", [D, DQ], f32r, kind="ExternalInput")
    Wk_d = nc.dram_tensor("Wk", [D, DQ], f32r, kind="ExternalInput")
    Wv_d = nc.dram_tensor("Wv", [D, DQ], f32r, kind="ExternalInput")
    Wo_d = nc.dram_tensor("Wo", [DQ, D], bf16, kind="ExternalInput")
    bq_d = nc.dram_tensor("bq", [DQ], f32, kind="ExternalInput")
    bk_d = nc.dram_tensor("bk", [DQ], f32, kind="ExternalInput")
    bv_d = nc.dram_tensor("bv", [DQ], f32r, kind="ExternalInput")
    bo_d = nc.dram_tensor("bo", [D], bf16, kind="ExternalInput")
    out_d = nc.dram_tensor("out", [L, D], f32, kind="ExternalOutput")

    xT_v = xT_d.ap().rearrange("(ko p) s -> p ko s", p=P)
    Wq_v = Wq_d.ap().rearrange("(ko p) d -> p ko d", p=P)
    Wk_v = Wk_d.ap().rearrange("(ko p) d -> p ko d", p=P)
    Wv_v = Wv_d.ap().rearrange("(ko p) d -> p ko d", p=P)
    Wo_v = Wo_d.ap().rearrange("(ko p) e -> p ko e", p=P)
    bq_v = bq_d.ap().rearrange("(c p) -> p c", p=P)
    bk_v = bk_d.ap().rearrange("(c p) -> p c", p=P)
    out_v = out_d.ap().rearrange("(ms p) e -> p ms e", p=P)

    with tile.TileContext(nc) as tc:
        with (
            tc.tile_pool(name="pp", bufs=1) as pp,
            tc.tile_pool(name="wp", bufs=1) as wp,
            tc.tile_pool(name="sp", bufs=1) as sp,
            tc.tile_pool(name="ps", bufs=1, space="PSUM") as ps,
        ):
            # ---- persistent tiles ----
            KT = pp.tile([P, DKC, L], f32r, name="KT")
            QT = pp.tile([P, DKC, L], f32r, name="QT")
            VA = pp.tile([P, ST, HPC, Dh + 1], f32r, name="VA")
            OT = pp.tile([P, WKO, L], bf16, name="OT")
            ones_f = pp.tile([P, P], f32, name="ones_f")
            ones_r = pp.tile([P, P], f32r, name="ones_r")
            ones_b = pp.tile([1, P], bf16, name="ones_b")
            nc.vector.memset(ones_f[:], 1.0)
            nc.vector.tensor_copy(ones_r[:], ones_f[:])
            nc.vector.tensor_copy(ones_b[:], ones_f[0:1, :])
            nc.vector.tensor_copy(VA[:, :, :, Dh:Dh + 1],
                                  ones_f[:, 0:1].to_broadcast((P, ST, HPC, 1)))
            bqs = pp.tile([P, DKC], f32, name="bqs")
            bks = pp.tile([P, DKC], f32, name="bks")
            bvs = pp.tile([1, DQ], f32r, name="bvs")
            bos = pp.tile([1, D], bf16, name="bos")
            nc.sync.dma_start(bqs[:], bq_v)
            nc.sync.dma_start(bks[:], bk_v)
            nc.sync.dma_start(bvs[:], bv_d.ap()[None, :])
            nc.sync.dma_start(bos[:], bo_d.ap()[None, :])

            # Wv resident for phase A
            Wv_sb = wp.tile([P, KO, DQ], f32r, name="Wv_sb")
            nc.sync.dma_start(Wv_sb[:], Wv_v)

            # ---- phase A: projections ----
            for sc in range(NSC):
                xts = sp.tile([P, KO, 512], f32r, tag="xts", bufs=2,
                              name=f"xts{sc}")
                nc.sync.dma_start(xts[:], xT_v[:, :, sc * 512:(sc + 1) * 512])
                for dkc in range(DKC):
                    for W_v_, T_sb, b_sb, nm in (
                        (Wk_v, KT, bks, "k"),
                        (Wq_v, QT, bqs, "q"),
                    ):
                        wt = sp.tile([P, KO, P], f32r, tag=f"w{nm}", bufs=1,
                                     name=f"w{nm}{sc}_{dkc}")
                        nc.sync.dma_start(
                            wt[:], W_v_[:, :, dkc * P:(dkc + 1) * P])
                        pt = ps.tile([P, 512], f32, tag="b512", bufs=4,
                                     name=f"p{nm}{sc}_{dkc}")
                        for ko in range(KO):
                            nc.tensor.matmul(pt[:], lhsT=wt[:, ko, :],
                                             rhs=xts[:, ko, :],
                                             start=(ko == 0),
                                             stop=(ko == KO - 1))
                        nc.scalar.activation(
                            T_sb[:, dkc, sc * 512:(sc + 1) * 512], pt[:],
                            AF.Identity, bias=b_sb[:, dkc:dkc + 1])
                for ssub in range(4):
                    st = sc * 4 + ssub
                    pv = ps.tile([P, 512], f32, tag="b512", bufs=4,
                                 name=f"pv{st}")
                    for ko in range(KO):
                        nc.tensor.matmul(
                            pv[:, 0:DQ],
                            lhsT=xts[:, ko, ssub * P:(ssub + 1) * P],
                            rhs=Wv_sb[:, ko, :],
                            start=(ko == 0), stop=False)
                    nc.tensor.matmul(pv[:, 0:DQ], lhsT=ones_r[0:1, 0:P],
                                     rhs=bvs[0:1, :], start=False, stop=True)
                    nc.vector.tensor_copy(
                        VA[:, st, :, 0:Dh],
                        pv[:, 0:DQ].rearrange("p (h d) -> p h d", d=Dh))

            # ---- phase B: attention per head ----
            for h in range(HPC):
                half = Dh * (h % 2)
                dkc = h // 2
                for mcg in range(MC // 2):
                    ops = []
                    for mci in range(2):
                        op = ps.tile([P, 512], f32, tag="b512", bufs=4,
                                     name=f"op{h}_{mcg}_{mci}")
                        ops.append(op)
                    for st in range(ST):
                        spt = ps.tile([P, 1024], f32, tag="b1024", bufs=2,
                                      name=f"sp{h}_{mcg}_{st}")
                        for mci in range(2):
                            mc = mcg * 2 + mci
                            nc.tensor.matmul(
                                spt[:, mci * 512:(mci + 1) * 512],
                                lhsT=KT[half:half + Dh, dkc,
                                        st * P:(st + 1) * P],
                                rhs=QT[half:half + Dh, dkc,
                                       mc * 512:(mc + 1) * 512],
                                start=True, stop=True)
                        es = sp.tile([P, 1024], f32r, tag="es", bufs=2,
                                     name=f"es{h}_{mcg}_{st}")
                        nc.scalar.activation(es[:], spt[:], AF.Exp,
                                             scale=0.125)
                        for mci in range(2):
                            nc.tensor.matmul(
                                ops[mci][0:Dh + 1, :],
                                lhsT=VA[:, st, h, :],
                                rhs=es[:, mci * 512:(mci + 1) * 512],
                                start=(st == 0), stop=(st == ST - 1))
                    for mci in range(2):
                        mc = mcg * 2 + mci
                        op = ops[mci]
                        dn = sp.tile([1, 512], f32, tag="dn", bufs=2,
                                     name=f"dn{h}_{mc}")
                        nc.vector.tensor_copy(dn[:], op[Dh:Dh + 1, :])
                        nc.vector.reciprocal_approx_fast(dn[:], dn[:])
                        dnr = sp.tile([1, 512], f32r, tag="dnr", bufs=2,
                                      name=f"dnr{h}_{mc}")
                        nc.vector.tensor_copy(dnr[:], dn[:])
                        bp = ps.tile([Dh, 512], f32, tag="b1024", bufs=2,
                                     name=f"bp{h}_{mc}")
                        nc.tensor.matmul(bp[:], lhsT=ones_r[0:1, 0:Dh],
                                         rhs=dnr[0:1, :],
                                         start=True, stop=True)
                        ot = sp.tile([Dh, 512], f32, tag="ott", bufs=2,
                                     name=f"ot{h}_{mc}")
                        nc.vector.tensor_copy(ot[:], op[0:Dh, :])
                        nc.vector.tensor_tensor(
                            OT[half:half + Dh, dkc, mc * 512:(mc + 1) * 512],
                            ot[:], bp[:], mybir.AluOpType.mult)

            # ---- phase C: partial out-projection ----
            Wo_sb = wp.tile([P, WKO, D], bf16, name="Wo_sb")
            nc.sync.dma_start(Wo_sb[:], Wo_v)
            for ms in range(MS):
                for ec in range(EC):
                    pt = ps.tile([P, 512], f32, tag="b512", bufs=4,
                                 name=f"po{ms}_{ec}")
                    for ko in range(WKO):
                        nc.tensor.matmul(
                            pt[:], lhsT=OT[:, ko, ms * P:(ms + 1) * P],
                            rhs=Wo_sb[:, ko, ec * 512:(ec + 1) * 512],
                            start=(ko == 0), stop=False)
                    nc.tensor.matmul(pt[:], lhsT=ones_b[0:1, 0:P],
                                     rhs=bos[0:1, ec * 512:(ec + 1) * 512],
                                     start=False, stop=True)
                    os_ = sp.tile([P, 512], f32, tag="os", bufs=2,
                                  name=f"os{ms}_{ec}")
                    nc.vector.tensor_copy(os_[:], pt[:])
                    nc.sync.dma_start(out_v[:, ms, ec * 512:(ec + 1) * 512],
                                      os_[:])

    nc.compile()
    return nc


# ---------------------------------------------------------------------------

_NC_CACHE = {}


def _get_nc():
    if "nc" not in _NC_CACHE:
        _NC_CACHE["nc"] = build_nc()
    return _NC_CACHE["nc"]


def _install_ntff_hook():
    """Provide antenv.axon_hooks (absent in this image) so trace=True can
    capture NTFF profiles for timing."""
    if "antenv.axon_hooks" in sys.modules:
        return
    mod = types.ModuleType("antenv.axon_hooks")
    holder = [None]
    mod.set_axon_ntff_profile_hook = lambda hk: holder.__setitem__(0, hk)
    mod.get_axon_ntff_profile_hook = lambda: holder[0]
    sys.modules["antenv.axon_hooks"] = mod
    import antenv

    antenv.axon_hooks = mod
    try:
        from trn_agent_boot.trn_boot import _ntff_profile_via_ctypes

        mod.set_axon_ntff_profile_hook(
            _ntff_profile_via_ctypes("/opt/axon/libaxon_pjrt.so"))
    except Exception:
        pass


def _make_in_maps(x, Wq, bq, Wk, bk, Wv, bv, Wo, bo):
    import ml_dtypes

    NB, L, D = x.shape          # 4, 2048, 1024
    DQ = D // 2                 # head-group width (8 heads x 64)
    in_maps = []
    for c in range(N_CORES):
        n, g = c % 4, c // 4
        sl = slice(g * DQ, (g + 1) * DQ)
        in_maps.append({
            "xT": np.ascontiguousarray(x[n].T).astype(np.float32),
            "Wq": np.ascontiguousarray(Wq[:, sl]).astype(np.float32),
            "Wk": np.ascontiguousarray(Wk[:, sl]).astype(np.float32),
            "Wv": np.ascontiguousarray(Wv[:, sl]).astype(np.float32),
            "Wo": np.ascontiguousarray(Wo[sl, :]).astype(ml_dtypes.bfloat16),
            "bq": np.ascontiguousarray(bq[sl]).astype(np.float32),
            "bk": np.ascontiguousarray(bk[sl]).astype(np.float32),
            "bv": np.ascontiguousarray(bv[sl]).astype(np.float32),
            "bo": (bo if g == 0 else np.zeros_like(bo)).astype(
                ml_dtypes.bfloat16),
        })
    return in_maps


def run_sharded(inputs, trace=False):
    """Run the SPMD kernel on the full inputs. Returns (output, exec_time_ns)."""
    nc = _get_nc()
    if trace:
        _install_ntff_hook()
    in_maps = _make_in_maps(**inputs)
    res = run_bass_kernel_spmd(nc, in_maps, list(range(N_CORES)), trace=trace)
    outs = [res.results[c]["out"] for c in range(N_CORES)]
    full = np.stack([outs[n] + outs[n + 4] for n in range(4)], axis=0)
    return full.astype(np.float32), res.exec_time_ns


def kernel(**inputs):
    out, _ = run_sharded(inputs, trace=False)
    return out


# revision 4
# speedup vs baseline: 1.1939x; 1.1939x over previous
"""Multi-head attention layer on 8 Trainium2 NeuronCores.

Reference (per batch n):
    Q = x@Wq + bq; K = x@Wk + bk; V = x@Wv + bv       (per-head split, Dh=64)
    out = softmax(Q K^T / sqrt(Dh)) V  -> concat heads -> @Wo + bo

Sharding: 2 head-groups (tensor parallel) x 4 batches (data parallel) = 8
cores. Core c handles batch c%4 and heads [8*(c//4), 8*(c//4)+8). Each core
computes a partial output projection with its Wo row-block; the host sums
the two head-group partials per batch (the only cross-core reduction).

Per-core kernel (all matmuls in float32r = TF32-like, fp32 accumulate;
output projection in bf16):
  A) projections: K^T,Q^T in [d_head(part), seq] layout, V in [seq(part),
     d_head] layout with an appended ones column (gives softmax denominators
     for free), streaming x^T per 512-column chunk.
  B) attention per head: S^T = K Q^T on TensorE (row-group 0/64 per head
     parity), exp via ScalarE over [128,1024] PSUM tiles (scale=1/8, no max
     subtraction -- scores are O(1) by construction), O^T += V_aug^T exp(S^T)
     accumulated over 16 seq-tiles, then normalize by the broadcast
     reciprocal of the denominator row (K=1 ones matmul broadcast).
  C) out^T partial = O_norm^T-contracted with Wo rows + bo (g==0 only).

Self-contained: hardcodes shapes for x:[4,2048,1024], d_model=1024, 16 heads.
"""

import sys
import types
import contextlib

import numpy as np

import concourse.bass as bass
import concourse.mybir as mybir
import concourse.tile as tile
from concourse import bacc
from concourse.bass_utils import run_bass_kernel_spmd

f32 = mybir.dt.float32
f32r = mybir.dt.float32r
bf16 = mybir.dt.bfloat16
AF = mybir.ActivationFunctionType

N_CORES = 8
P = 128

# ---------------------------------------------------------------------------


def build_nc(L=2048, D=1024, HPC=8, Dh=64):
    """Build the per-core Bass graph (SPMD: same graph, per-core shards)."""
    KO = D // P          # k-tiles over d_model
    DQ = HPC * Dh        # local projected dim
    DKC = DQ // P        # 128-row chunks of DQ
    NSC = L // 512       # 512-wide seq chunks
    ST = L // P          # 128-row seq tiles
    MC = L // 512        # 512-wide m chunks
    WKO = DQ // P        # k-tiles for out-proj contraction
    EC = D // 512        # 512-wide out chunks
    MS = L // P          # 128-row out row-tiles
    assert MC % 2 == 0 and HPC % 2 == 0

    nc = bacc.Bacc("TRN2", target_bir_lowering=False, debug=False,
                   num_devices=N_CORES)

    xT_d = nc.dram_tensor("xT", [D, L], bf16, kind="ExternalInput")
    Wq_d = nc.dram_tensor("Wq", [D, DQ], bf16, kind="ExternalInput")
    Wk_d = nc.dram_tensor("Wk", [D, DQ], bf16, kind="ExternalInput")
    Wv_d = nc.dram_tensor("Wv", [D, DQ], bf16, kind="ExternalInput")
    Wo_d = nc.dram_tensor("Wo", [DQ, D], bf16, kind="ExternalInput")
    bq_d = nc.dram_tensor("bq", [DQ], f32, kind="ExternalInput")
    bk_d = nc.dram_tensor("bk", [DQ], f32, kind="ExternalInput")
    bv_d = nc.dram_tensor("bv", [DQ], bf16, kind="ExternalInput")
    bo_d = nc.dram_tensor("bo", [D], bf16, kind="ExternalInput")
    out_d = nc.dram_tensor("out", [L, D], f32, kind="ExternalOutput")

    xT_v = xT_d.ap().rearrange("(ko p) s -> p ko s", p=P)
    Wq_v = Wq_d.ap().rearrange("(ko p) d -> p ko d", p=P)
    Wk_v = Wk_d.ap().rearrange("(ko p) d -> p ko d", p=P)
    Wv_v = Wv_d.ap().rearrange("(ko p) d -> p ko d", p=P)
    Wo_v = Wo_d.ap().rearrange("(ko p) e -> p ko e", p=P)
    bq_v = bq_d.ap().rearrange("(c p) -> p c", p=P)
    bk_v = bk_d.ap().rearrange("(c p) -> p c", p=P)
    out_v = out_d.ap().rearrange("(ms p) e -> p ms e", p=P)

    with tile.TileContext(nc) as tc:
        with (
            tc.tile_pool(name="pp", bufs=1) as pp,
            tc.tile_pool(name="wp", bufs=1) as wp,
            tc.tile_pool(name="sp", bufs=1) as sp,
            tc.tile_pool(name="ps", bufs=1, space="PSUM") as ps,
        ):
            # ---- persistent tiles ----
            KT = pp.tile([P, DKC, L], bf16, name="KT")
            QT = pp.tile([P, DKC, L], bf16, name="QT")
            VA = pp.tile([P, ST, HPC, Dh + 1], bf16, name="VA")
            OT = pp.tile([P, WKO, L], bf16, name="OT")
            ones_f = pp.tile([P, P], f32, name="ones_f")
            ones_r = pp.tile([P, P], f32r, name="ones_r")
            ones_b = pp.tile([P, P], bf16, name="ones_b")
            nc.vector.memset(ones_f[:], 1.0)
            nc.vector.tensor_copy(ones_r[:], ones_f[:])
            nc.vector.tensor_copy(ones_b[:], ones_f[:])
            nc.vector.tensor_copy(VA[:, :, :, Dh:Dh + 1],
                                  ones_f[:, 0:1].to_broadcast((P, ST, HPC, 1)))
            bqs = pp.tile([P, DKC], f32, name="bqs")
            bks = pp.tile([P, DKC], f32, name="bks")
            bvs = pp.tile([1, DQ], bf16, name="bvs")
            bos = pp.tile([1, D], bf16, name="bos")
            nc.sync.dma_start(bqs[:], bq_v)
            nc.sync.dma_start(bks[:], bk_v)
            nc.sync.dma_start(bvs[:], bv_d.ap()[None, :])
            nc.sync.dma_start(bos[:], bo_d.ap()[None, :])

            # Wv resident for phase A
            Wv_sb = wp.tile([P, KO, DQ], bf16, name="Wv_sb")
            nc.sync.dma_start(Wv_sb[:], Wv_v)

            # ---- phase A: projections ----
            for sc in range(NSC):
                xts = sp.tile([P, KO, 512], bf16, tag="xts", bufs=2,
                              name=f"xts{sc}")
                nc.sync.dma_start(xts[:], xT_v[:, :, sc * 512:(sc + 1) * 512])
                for dkc in range(DKC):
                    for W_v_, T_sb, b_sb, nm in (
                        (Wk_v, KT, bks, "k"),
                        (Wq_v, QT, bqs, "q"),
                    ):
                        wt = sp.tile([P, KO, P], bf16, tag=f"w{nm}", bufs=2,
                                     name=f"w{nm}{sc}_{dkc}")
                        nc.sync.dma_start(
                            wt[:], W_v_[:, :, dkc * P:(dkc + 1) * P])
                        pt = ps.tile([P, 512], f32, tag="b512", bufs=4,
                                     name=f"p{nm}{sc}_{dkc}")
                        for ko in range(KO):
                            nc.tensor.matmul(pt[:], lhsT=wt[:, ko, :],
                                             rhs=xts[:, ko, :],
                                             start=(ko == 0),
                                             stop=(ko == KO - 1))
                        nc.scalar.activation(
                            T_sb[:, dkc, sc * 512:(sc + 1) * 512], pt[:],
                            AF.Identity, bias=b_sb[:, dkc:dkc + 1])
                for ssub in range(4):
                    st = sc * 4 + ssub
                    pv = ps.tile([P, 512], f32, tag="b512", bufs=4,
                                 name=f"pv{st}")
                    for ko in range(KO):
                        nc.tensor.matmul(
                            pv[:, 0:DQ],
                            lhsT=xts[:, ko, ssub * P:(ssub + 1) * P],
                            rhs=Wv_sb[:, ko, :],
                            start=(ko == 0), stop=False)
                    nc.tensor.matmul(pv[:, 0:DQ], lhsT=ones_b[0:1, 0:P],
                                     rhs=bvs[0:1, :], start=False, stop=True)
                    nc.vector.tensor_copy(
                        VA[:, st, :, 0:Dh],
                        pv[:, 0:DQ].rearrange("p (h d) -> p h d", d=Dh))

            # ---- phase B: attention per head ----
            for h in range(HPC):
                half = Dh * (h % 2)
                dkc = h // 2
                for mcg in range(MC // 2):
                    ops = []
                    for mci in range(2):
                        op = ps.tile([P, 512], f32, tag="b512", bufs=4,
                                     name=f"op{h}_{mcg}_{mci}")
                        ops.append(op)
                    for st in range(ST):
                        spt = ps.tile([P, 1024], f32, tag="b1024", bufs=2,
                                      name=f"sp{h}_{mcg}_{st}")
                        for mci in range(2):
                            mc = mcg * 2 + mci
                            nc.tensor.matmul(
                                spt[:, mci * 512:(mci + 1) * 512],
                                lhsT=KT[half:half + Dh, dkc,
                                        st * P:(st + 1) * P],
                                rhs=QT[half:half + Dh, dkc,
                                       mc * 512:(mc + 1) * 512],
                                start=True, stop=True)
                        es = sp.tile([P, 1024], bf16, tag="es", bufs=3,
                                     name=f"es{h}_{mcg}_{st}")
                        nc.scalar.activation(es[:], spt[:], AF.Exp,
                                             scale=0.125)
                        for mci in range(2):
                            nc.tensor.matmul(
                                ops[mci][0:Dh + 1, :],
                                lhsT=VA[:, st, h, :],
                                rhs=es[:, mci * 512:(mci + 1) * 512],
                                start=(st == 0), stop=(st == ST - 1))
                    for mci in range(2):
                        mc = mcg * 2 + mci
                        op = ops[mci]
                        dn = sp.tile([1, 512], f32, tag="dn", bufs=2,
                                     name=f"dn{h}_{mc}")
                        nc.vector.tensor_copy(dn[:], op[Dh:Dh + 1, :])
                        nc.vector.reciprocal_approx_fast(dn[:], dn[:])
                        dnr = sp.tile([1, 512], f32r, tag="dnr", bufs=2,
                                      name=f"dnr{h}_{mc}")
                        nc.vector.tensor_copy(dnr[:], dn[:])
                        bp = ps.tile([Dh, 512], f32, tag="b1024", bufs=2,
                                     name=f"bp{h}_{mc}")
                        nc.tensor.matmul(bp[:], lhsT=ones_r[0:1, 0:Dh],
                                         rhs=dnr[0:1, :],
                                         start=True, stop=True)
                        ot = sp.tile([Dh, 512], f32, tag="ott", bufs=2,
                                     name=f"ot{h}_{mc}")
                        nc.vector.tensor_copy(ot[:], op[0:Dh, :])
                        nc.vector.tensor_tensor(
                            OT[half:half + Dh, dkc, mc * 512:(mc + 1) * 512],
                            ot[:], bp[:], mybir.AluOpType.mult)

            # ---- phase C: partial out-projection ----
            Wo_sb = wp.tile([P, WKO, D], bf16, name="Wo_sb")
            nc.sync.dma_start(Wo_sb[:], Wo_v)
            for ms in range(MS):
                for ec in range(EC):
                    pt = ps.tile([P, 512], f32, tag="b512", bufs=4,
                                 name=f"po{ms}_{ec}")
                    for ko in range(WKO):
                        nc.tensor.matmul(
                            pt[:], lhsT=OT[:, ko, ms * P:(ms + 1) * P],
                            rhs=Wo_sb[:, ko, ec * 512:(ec + 1) * 512],
                            start=(ko == 0), stop=False)
                    nc.tensor.matmul(pt[:], lhsT=ones_b[0:1, 0:P],
                                     rhs=bos[0:1, ec * 512:(ec + 1) * 512],
                                     start=False, stop=True)
                    os_ = sp.tile([P, 512], f32, tag="os", bufs=2,
                                  name=f"os{ms}_{ec}")
                    nc.vector.tensor_copy(os_[:], pt[:])
                    nc.sync.dma_start(out_v[:, ms, ec * 512:(ec + 1) * 512],
                                      os_[:])

    nc.compile()
    return nc


# ---------------------------------------------------------------------------

_NC_CACHE = {}


def _get_nc():
    if "nc" not in _NC_CACHE:
        _NC_CACHE["nc"] = build_nc()
    return _NC_CACHE["nc"]


def _install_ntff_hook():
    """Provide antenv.axon_hooks (absent in this image) so trace=True can
    capture NTFF profiles for timing."""
    if "antenv.axon_hooks" in sys.modules:
        return
    mod = types.ModuleType("antenv.axon_hooks")
    holder = [None]
    mod.set_axon_ntff_profile_hook = lambda hk: holder.__setitem__(0, hk)
    mod.get_axon_ntff_profile_hook = lambda: holder[0]
    sys.modules["antenv.axon_hooks"] = mod
    import antenv

    antenv.axon_hooks = mod
    try:
        from trn_agent_boot.trn_boot import _ntff_profile_via_ctypes

        mod.set_axon_ntff_profile_hook(
            _ntff_profile_via_ctypes("/opt/axon/libaxon_pjrt.so"))
    except Exception:
        pass


def _make_in_maps(x, Wq, bq, Wk, bk, Wv, bv, Wo, bo):
    import ml_dtypes

    NB, L, D = x.shape          # 4, 2048, 1024
    DQ = D // 2                 # head-group width (8 heads x 64)
    in_maps = []
    for c in range(N_CORES):
        n, g = c % 4, c // 4
        sl = slice(g * DQ, (g + 1) * DQ)
        in_maps.append({
            "xT": np.ascontiguousarray(x[n].T).astype(ml_dtypes.bfloat16),
            "Wq": np.ascontiguousarray(Wq[:, sl]).astype(ml_dtypes.bfloat16),
            "Wk": np.ascontiguousarray(Wk[:, sl]).astype(ml_dtypes.bfloat16),
            "Wv": np.ascontiguousarray(Wv[:, sl]).astype(ml_dtypes.bfloat16),
            "Wo": np.ascontiguousarray(Wo[sl, :]).astype(ml_dtypes.bfloat16),
            "bq": np.ascontiguousarray(bq[sl]).astype(np.float32),
            "bk": np.ascontiguousarray(bk[sl]).astype(np.float32),
            "bv": np.ascontiguousarray(bv[sl]).astype(ml_dtypes.bfloat16),
            "bo": (bo if g == 0 else np.zeros_like(bo)).astype(
                ml_dtypes.bfloat16),
        })
    return in_maps


def run_sharded(inputs, trace=False):
    """Run the SPMD kernel on the full inputs. Returns (output, exec_time_ns)."""
    nc = _get_nc()
    if trace:
        _install_ntff_hook()
    in_maps = _make_in_maps(**inputs)
    res = run_bass_kernel_spmd(nc, in_maps, list(range(N_CORES)), trace=trace)
    outs = [res.results[c]["out"] for c in range(N_CORES)]
    full = np.stack([outs[n] + outs[n + 4] for n in range(4)], axis=0)
    return full.astype(np.float32), res.exec_time_ns


def kernel(**inputs):
    out, _ = run_sharded(inputs, trace=False)
    return out


# revision 7
# speedup vs baseline: 1.4416x; 1.2075x over previous
"""Multi-head attention layer on 8 Trainium2 NeuronCores.

Reference (per batch n):
    Q = x@Wq + bq; K = x@Wk + bk; V = x@Wv + bv       (per-head split, Dh=64)
    out = softmax(Q K^T / sqrt(Dh)) V  -> concat heads -> @Wo + bo

Sharding: 2 head-groups (tensor parallel) x 4 batches (data parallel) = 8
cores. Core c handles batch c%4 and heads [8*(c//4), 8*(c//4)+8). Each core
computes a partial output projection with its Wo row-block; the host sums
the two head-group partials per batch (the only cross-core reduction).

Per-core kernel (all matmuls in float32r = TF32-like, fp32 accumulate;
output projection in bf16):
  A) projections: K^T,Q^T in [d_head(part), seq] layout, V in [seq(part),
     d_head] layout with an appended ones column (gives softmax denominators
     for free), streaming x^T per 512-column chunk.
  B) attention per head: S^T = K Q^T on TensorE (row-group 0/64 per head
     parity), exp via ScalarE over [128,1024] PSUM tiles (scale=1/8, no max
     subtraction -- scores are O(1) by construction), O^T += V_aug^T exp(S^T)
     accumulated over 16 seq-tiles, then normalize by the broadcast
     reciprocal of the denominator row (K=1 ones matmul broadcast).
  C) out^T partial = O_norm^T-contracted with Wo rows + bo (g==0 only).

Self-contained: hardcodes shapes for x:[4,2048,1024], d_model=1024, 16 heads.
"""

import sys
import types
import contextlib

import numpy as np

import concourse.bass as bass
import concourse.mybir as mybir
import concourse.tile as tile
from concourse import bacc
from concourse.bass_utils import run_bass_kernel_spmd

f32 = mybir.dt.float32
f32r = mybir.dt.float32r
bf16 = mybir.dt.bfloat16
AF = mybir.ActivationFunctionType

N_CORES = 8
P = 128

# ---------------------------------------------------------------------------


def build_nc(L=2048, D=1024, HPC=8, Dh=64):
    """Build the per-core Bass graph (SPMD: same graph, per-core shards)."""
    KO = D // P          # k-tiles over d_model
    DQ = HPC * Dh        # local projected dim
    DKC = DQ // P        # 128-row chunks of DQ
    NSC = L // 512       # 512-wide seq chunks
    ST = L // P          # 128-row seq tiles
    MC = L // 512        # 512-wide m chunks
    WKO = DQ // P        # k-tiles for out-proj contraction
    EC = D // 512        # 512-wide out chunks
    MS = L // P          # 128-row out row-tiles
    assert MC % 2 == 0 and HPC % 2 == 0

    nc = bacc.Bacc("TRN2", target_bir_lowering=False, debug=False,
                   num_devices=N_CORES)

    xT_d = nc.dram_tensor("xT", [D, L], bf16, kind="ExternalInput")
    Wq_d = nc.dram_tensor("Wq", [D, DQ], bf16, kind="ExternalInput")
    Wk_d = nc.dram_tensor("Wk", [D, DQ], bf16, kind="ExternalInput")
    Wv_d = nc.dram_tensor("Wv", [D, DQ], bf16, kind="ExternalInput")
    Wo_d = nc.dram_tensor("Wo", [DQ, D], bf16, kind="ExternalInput")
    bq_d = nc.dram_tensor("bq", [DQ], bf16, kind="ExternalInput")
    bk_d = nc.dram_tensor("bk", [DQ], bf16, kind="ExternalInput")
    bv_d = nc.dram_tensor("bv", [DQ], bf16, kind="ExternalInput")
    bo_d = nc.dram_tensor("bo", [D], bf16, kind="ExternalInput")
    out_d = nc.dram_tensor("out", [L, D], f32, kind="ExternalOutput")

    xT_v = xT_d.ap().rearrange("(ko p) s -> p ko s", p=P)
    Wq_v = Wq_d.ap().rearrange("(ko p) d -> p ko d", p=P)
    Wk_v = Wk_d.ap().rearrange("(ko p) d -> p ko d", p=P)
    Wv_v = Wv_d.ap().rearrange("(ko p) d -> p ko d", p=P)
    Wo_v = Wo_d.ap().rearrange("(ko p) e -> p ko e", p=P)
    out_v = out_d.ap().rearrange("(ms p) e -> p ms e", p=P)

    with tile.TileContext(nc) as tc:
        with (
            tc.tile_pool(name="pp", bufs=1) as pp,
            tc.tile_pool(name="wp", bufs=1) as wp,
            tc.tile_pool(name="sp", bufs=1) as sp,
            tc.tile_pool(name="ps", bufs=1, space="PSUM") as ps,
        ):
            # ---- persistent tiles ----
            KT = pp.tile([P, HPC, L], bf16, name="KT")
            QT = pp.tile([P, HPC, L], bf16, name="QT")
            VA = pp.tile([P, ST, HPC, Dh + 1], bf16, name="VA")
            OT = pp.tile([P, WKO, L], bf16, name="OT")
            ones_f = pp.tile([P, P], f32, name="ones_f")
            ones_r = pp.tile([P, P], f32r, name="ones_r")
            ones_b = pp.tile([1, 512], bf16, name="ones_b")
            nc.vector.memset(ones_f[:], 1.0)
            nc.vector.tensor_copy(ones_r[:], ones_f[:])
            nc.vector.memset(ones_b[:], 1.0)
            nc.vector.tensor_copy(VA[:, :, :, Dh:Dh + 1],
                                  ones_f[:, 0:1].to_broadcast((P, ST, HPC, 1)))
            bqs = pp.tile([1, DQ], bf16, name="bqs")
            bks = pp.tile([1, DQ], bf16, name="bks")
            bvs = pp.tile([1, DQ], bf16, name="bvs")
            bos = pp.tile([1, D], bf16, name="bos")
            nc.sync.dma_start(bqs[:], bq_d.ap()[None, :])
            nc.sync.dma_start(bks[:], bk_d.ap()[None, :])
            nc.vector.memset(KT[64:128, :, :], 0.0)
            nc.vector.memset(QT[64:128, :, :], 0.0)
            nc.sync.dma_start(bvs[:], bv_d.ap()[None, :])
            nc.sync.dma_start(bos[:], bo_d.ap()[None, :])

            # Wv resident for phase A
            Wv_sb = wp.tile([P, KO, DQ], bf16, name="Wv_sb")
            nc.sync.dma_start(Wv_sb[:], Wv_v)

            # ---- phase A: projections ----
            for sc in range(NSC):
                xts = sp.tile([P, KO, 512], bf16, tag="xts", bufs=2,
                              name=f"xts{sc}")
                nc.sync.dma_start(xts[:], xT_v[:, :, sc * 512:(sc + 1) * 512])
                for dkc in range(DKC):
                    for W_v_, T_sb, b_sb, nm in (
                        (Wk_v, KT, bks, "k"),
                        (Wq_v, QT, bqs, "q"),
                    ):
                        wt = sp.tile([P, KO, P], bf16, tag=f"w{nm}", bufs=2,
                                     name=f"w{nm}{sc}_{dkc}")
                        nc.sync.dma_start(
                            wt[:], W_v_[:, :, dkc * P:(dkc + 1) * P])
                        pt = ps.tile([P, 512], f32, tag="b512", bufs=4,
                                     name=f"p{nm}{sc}_{dkc}")
                        for ko in range(KO):
                            nc.tensor.matmul(pt[:], lhsT=wt[:, ko, :],
                                             rhs=xts[:, ko, :],
                                             start=(ko == 0), stop=False)
                        nc.tensor.matmul(
                            pt[:], lhsT=b_sb[0:1, dkc * P:(dkc + 1) * P],
                            rhs=ones_b[0:1, 0:512], start=False, stop=True)
                        ssl = slice(sc * 512, (sc + 1) * 512)
                        nc.scalar.copy(T_sb[0:64, 2 * dkc, ssl], pt[0:64, :])
                        nc.vector.tensor_copy(
                            T_sb[0:64, 2 * dkc + 1, ssl], pt[64:128, :])
                for ssub in range(4):
                    st = sc * 4 + ssub
                    pv = ps.tile([P, 512], f32, tag="b512", bufs=4,
                                 name=f"pv{st}")
                    for ko in range(KO):
                        nc.tensor.matmul(
                            pv[:, 0:DQ],
                            lhsT=xts[:, ko, ssub * P:(ssub + 1) * P],
                            rhs=Wv_sb[:, ko, :],
                            start=(ko == 0), stop=False)
                    nc.tensor.matmul(pv[:, 0:DQ], lhsT=ones_b[0:1, 0:P],
                                     rhs=bvs[0:1, :], start=False, stop=True)
                    nc.vector.tensor_copy(
                        VA[:, st, :, 0:Dh],
                        pv[:, 0:DQ].rearrange("p (h d) -> p h d", d=Dh))

            # ---- phase B: attention per head ----
            for h in range(HPC):
                half = Dh * (h % 2)
                dkc = h // 2
                for mcg in range(MC // 2):  # noqa
                    ops = []
                    for mci in range(2):
                        op = ps.tile([P, 512], f32, tag="b512", bufs=4,
                                     name=f"op{h}_{mcg}_{mci}")
                        ops.append(op)
                    for st in range(ST):
                        spt = ps.tile([P, 1024], f32, tag="b1024", bufs=2,
                                      name=f"sp{h}_{mcg}_{st}")
                        for mci in range(2):
                            mc = mcg * 2 + mci
                            nc.tensor.matmul(
                                spt[:, mci * 512:(mci + 1) * 512],
                                lhsT=KT[:, h, st * P:(st + 1) * P],
                                rhs=QT[:, h, mc * 512:(mc + 1) * 512],
                                start=True, stop=True)
                        es = sp.tile([P, 1024], bf16, tag="es", bufs=3,
                                     name=f"es{h}_{mcg}_{st}")
                        nc.scalar.activation(es[:], spt[:], AF.Exp,
                                             scale=0.125)
                        for mci in range(2):
                            nc.tensor.matmul(
                                ops[mci][0:Dh + 1, :],
                                lhsT=VA[:, st, h, :],
                                rhs=es[:, mci * 512:(mci + 1) * 512],
                                start=(st == 0), stop=(st == ST - 1))
                    for mci in range(2):
                        mc = mcg * 2 + mci
                        op = ops[mci]
                        dn = sp.tile([1, 512], f32, tag="dn", bufs=2,
                                     name=f"dn{h}_{mc}")
                        nc.vector.tensor_copy(dn[:], op[Dh:Dh + 1, :])
                        nc.vector.reciprocal_approx_fast(dn[:], dn[:])
                        dnr = sp.tile([1, 512], f32r, tag="dnr", bufs=2,
                                      name=f"dnr{h}_{mc}")
                        nc.vector.tensor_copy(dnr[:], dn[:])
                        bp = ps.tile([Dh, 512], f32, tag="b1024", bufs=2,
                                     name=f"bp{h}_{mc}")
                        nc.tensor.matmul(bp[:], lhsT=ones_r[0:1, 0:Dh],
                                         rhs=dnr[0:1, :],
                                         start=True, stop=True)
                        ot = sp.tile([Dh, 512], f32, tag="ott", bufs=2,
                                     name=f"ot{h}_{mc}")
                        nc.vector.tensor_copy(ot[:], op[0:Dh, :])
                        nc.vector.tensor_tensor(
                            OT[half:half + Dh, dkc, mc * 512:(mc + 1) * 512],
                            ot[:], bp[:], mybir.AluOpType.mult)

            # ---- phase C: partial out-projection ----
            Wo_sb = wp.tile([P, WKO, D], bf16, name="Wo_sb")
            nc.sync.dma_start(Wo_sb[:], Wo_v)
            for ms in range(MS):
                for ec in range(EC):
                    pt = ps.tile([P, 512], f32, tag="b512", bufs=4,
                                 name=f"po{ms}_{ec}")
                    for ko in range(WKO):
                        nc.tensor.matmul(
                            pt[:], lhsT=OT[:, ko, ms * P:(ms + 1) * P],
                            rhs=Wo_sb[:, ko, ec * 512:(ec + 1) * 512],
                            start=(ko == 0), stop=False)
                    nc.tensor.matmul(pt[:], lhsT=ones_b[0:1, 0:P],
                                     rhs=bos[0:1, ec * 512:(ec + 1) * 512],
                                     start=False, stop=True)
                    os_ = sp.tile([P, 512], f32, tag="os", bufs=2,
                                  name=f"os{ms}_{ec}")
                    nc.vector.tensor_copy(os_[:], pt[:])
                    nc.sync.dma_start(out_v[:, ms, ec * 512:(ec + 1) * 512],
                                      os_[:])

    nc.compile()
    return nc


# ---------------------------------------------------------------------------

_NC_CACHE = {}


def _get_nc():
    if "nc" not in _NC_CACHE:
        _NC_CACHE["nc"] = build_nc()
    return _NC_CACHE["nc"]


def _install_ntff_hook():
    """Provide antenv.axon_hooks (absent in this image) so trace=True can
    capture NTFF profiles for timing."""
    if "antenv.axon_hooks" in sys.modules:
        return
    mod = types.ModuleType("antenv.axon_hooks")
    holder = [None]
    mod.set_axon_ntff_profile_hook = lambda hk: holder.__setitem__(0, hk)
    mod.get_axon_ntff_profile_hook = lambda: holder[0]
    sys.modules["antenv.axon_hooks"] = mod
    import antenv

    antenv.axon_hooks = mod
    try:
        from trn_agent_boot.trn_boot import _ntff_profile_via_ctypes

        mod.set_axon_ntff_profile_hook(
            _ntff_profile_via_ctypes("/opt/axon/libaxon_pjrt.so"))
    except Exception:
        pass


def _make_in_maps(x, Wq, bq, Wk, bk, Wv, bv, Wo, bo):
    import ml_dtypes

    NB, L, D = x.shape          # 4, 2048, 1024
    DQ = D // 2                 # head-group width (8 heads x 64)
    in_maps = []
    for c in range(N_CORES):
        n, g = c % 4, c // 4
        sl = slice(g * DQ, (g + 1) * DQ)
        in_maps.append({
            "xT": np.ascontiguousarray(x[n].T).astype(ml_dtypes.bfloat16),
            "Wq": np.ascontiguousarray(Wq[:, sl]).astype(ml_dtypes.bfloat16),
            "Wk": np.ascontiguousarray(Wk[:, sl]).astype(ml_dtypes.bfloat16),
            "Wv": np.ascontiguousarray(Wv[:, sl]).astype(ml_dtypes.bfloat16),
            "Wo": np.ascontiguousarray(Wo[sl, :]).astype(ml_dtypes.bfloat16),
            "bq": np.ascontiguousarray(bq[sl]).astype(ml_dtypes.bfloat16),
            "bk": np.ascontiguousarray(bk[sl]).astype(ml_dtypes.bfloat16),
            "bv": np.ascontiguousarray(bv[sl]).astype(ml_dtypes.bfloat16),
            "bo": (bo if g == 0 else np.zeros_like(bo)).astype(
                ml_dtypes.bfloat16),
        })
    return in_maps


def run_sharded(inputs, trace=False):
    """Run the SPMD kernel on the full inputs. Returns (output, exec_time_ns)."""
    nc = _get_nc()
    if trace:
        _install_ntff_hook()
    in_maps = _make_in_maps(**inputs)
    res = run_bass_kernel_spmd(nc, in_maps, list(range(N_CORES)), trace=trace)
    outs = [res.results[c]["out"] for c in range(N_CORES)]
    full = np.stack([outs[n] + outs[n + 4] for n in range(4)], axis=0)
    return full.astype(np.float32), res.exec_time_ns


def kernel(**inputs):
    out, _ = run_sharded(inputs, trace=False)
    return out
